# revision 1
# baseline (speedup 1.0000x reference)
"""Causal self-attention with RoPE (B=2, T=2048, C=2048, 16 heads) on 8 TRN2 cores.

Sharding: data-parallel over batch (2) x tensor-parallel over head groups
(16 heads -> 4 groups of 4), Megatron-style. Core c handles batch c//4 and
head group c%4: fused QKV projection (column-parallel slice of w_attn), RoPE,
causal attention for its 4 heads, and the row-parallel c_proj partial product.
No on-device collectives: the c_proj all-reduce is a host-side sum of the 4
partials per batch element.

Per-core device pipeline (all matmuls bf16, fp32 PSUM accumulation):
  - qT/kT produced directly in [d, t] layout (weights as stationary operand,
    host-transposed xT as moving operand); v in natural [t, d] layout.
  - RoPE applied in [d, t] layout. The rotation pair-shuffle is turned into a
    partition half-swap by host-permuting the q/k weight columns
    (even dims first, odd dims second); the swap itself is 2 SBUF-to-SBUF
    DMAs, then 2 muls + 1 add against host-precomputed cos/sin tables.
  - Scores computed transposed ([k, q]) so the exp() output feeds the PV
    matmul directly (no PE transposes anywhere in attention).
  - Lazy softmax: no max subtraction (scores are O(5) here; exp is safe in
    fp32/bf16 range), denominator Z from a ones-vector matmul, and the
    1/Z normalization applied to the small PV output via DVE reciprocal +
    K=1 broadcast matmul.
  - Causal masking: score tiles strictly above the diagonal are never
    computed; diagonal 128x128 subtiles get a 0/1 triangular mask multiply.
"""

import sys

if "/opt/trn_rl_repo" not in sys.path:
    sys.path.insert(0, "/opt/trn_rl_repo")

import numpy as np
import ml_dtypes

import concourse.bacc as bacc
import concourse.tile as tile
from concourse import mybir
from concourse.bass_utils import run_bass_kernel_spmd

BF16 = ml_dtypes.bfloat16
F32 = mybir.dt.float32
BF = mybir.dt.bfloat16

B, T, C = 2, 2048, 2048
N_HEAD = 16
D = 128
N_CORES = 8
GROUPS = 4            # head groups (tensor-parallel)
HPC = N_HEAD // GROUPS  # heads per core = 4
DV = HPC * D          # per-core qkv width = 512
ROPE_THETA = 10000.0


def _deinterleave_perm():
    return np.concatenate([np.arange(0, D, 2), np.arange(1, D, 2)])


def _rope_tables(start_index):
    j = np.arange(D // 2, dtype=np.float64)
    inv_freq = 1.0 / (ROPE_THETA ** (2.0 * j / D))
    pos = np.arange(T, dtype=np.float64) + float(start_index)
    ang = np.outer(inv_freq, pos)  # [64, T]
    cos, sin = np.cos(ang), np.sin(ang)
    cosf = np.concatenate([cos, cos], axis=0).astype(np.float32)
    sins = np.concatenate([-sin, sin], axis=0).astype(np.float32)
    return cosf, sins


def make_core_inputs(x_b, wq_raw, wk_raw, wv_raw, wp_raw, start_index):
    """Host prep for one core: permute/cast weights, build tables."""
    perm = _deinterleave_perm()

    def permute_heads(w):
        w = w.reshape(C, HPC, D)
        return np.ascontiguousarray(w[:, :, perm].reshape(C, HPC * D))

    cosf, sins = _rope_tables(start_index)
    tri = (np.arange(128)[:, None] <= np.arange(128)[None, :]).astype(np.float32)
    return {
        "xT": np.ascontiguousarray(x_b.T).astype(BF16),
        "wq": permute_heads(wq_raw).astype(BF16),
        "wk": permute_heads(wk_raw).astype(BF16),
        "wv": np.ascontiguousarray(wv_raw).astype(BF16),
        "wp": np.ascontiguousarray(wp_raw).astype(BF16),
        "cosf": cosf.astype(BF16),
        "sins": sins.astype(BF16),
        "tri": tri.astype(BF16),
    }


def build_nc(debug=False):
    """Build + bass-compile the per-core program (same on all 8 cores)."""
    n_tc = T // 512   # 4
    n_ct = C // 128   # 16
    n_kt = T // 128   # 16
    n_qc = T // 512   # 4
    SCALE = 1.0 / float(np.sqrt(D))

    nc = bacc.Bacc(None, target_bir_lowering=False, debug=debug)

    xT = nc.declare_dram_parameter("xT", [C, T], BF, isOutput=False)
    wq = nc.declare_dram_parameter("wq", [C, DV], BF, isOutput=False)
    wk = nc.declare_dram_parameter("wk", [C, DV], BF, isOutput=False)
    wv = nc.declare_dram_parameter("wv", [C, DV], BF, isOutput=False)
    wp = nc.declare_dram_parameter("wp", [DV, C], BF, isOutput=False)
    cosf = nc.declare_dram_parameter("cosf", [128, T], BF, isOutput=False)
    sins = nc.declare_dram_parameter("sins", [128, T], BF, isOutput=False)
    tri = nc.declare_dram_parameter("tri", [128, 128], BF, isOutput=False)
    y = nc.declare_dram_parameter("y", [T, C], F32, isOutput=True)

    with tile.TileContext(nc) as tc:
        with tc.tile_pool(name="const", bufs=1) as const, \
             tc.tile_pool(name="big", bufs=1) as big, \
             tc.tile_pool(name="work", bufs=3) as work:

            wq_sb = const.tile([128, n_ct, DV], BF)
            wk_sb = const.tile([128, n_ct, DV], BF)
            wv_sb = const.tile([128, n_ct, DV], BF)
            wp_sb = const.tile([128, HPC, C], BF)
            cosf_sb = const.tile([128, T], BF)
            sins_sb = const.tile([128, T], BF)
            tri_sb = const.tile([128, 128], BF)
            ones_bf = const.tile([128, 1], BF)
            ones_f = const.tile([1, 128], F32)

            qrot = big.tile([128, HPC, T], BF)
            krot = big.tile([128, HPC, T], BF)
            v_sb = big.tile([128, n_kt, DV], BF)
            attoT = big.tile([128, HPC, T], BF)

            for ct in range(n_ct):
                nc.sync.dma_start(out=wq_sb[:, ct, :], in_=wq[ct * 128:(ct + 1) * 128, :])
                nc.sync.dma_start(out=wk_sb[:, ct, :], in_=wk[ct * 128:(ct + 1) * 128, :])
                nc.sync.dma_start(out=wv_sb[:, ct, :], in_=wv[ct * 128:(ct + 1) * 128, :])
            for h in range(HPC):
                nc.sync.dma_start(out=wp_sb[:, h, :], in_=wp[h * 128:(h + 1) * 128, :])
            nc.sync.dma_start(out=cosf_sb, in_=cosf[:, :])
            nc.sync.dma_start(out=sins_sb, in_=sins[:, :])
            nc.sync.dma_start(out=tri_sb, in_=tri[:, :])
            nc.vector.memset(ones_bf, 1.0)
            nc.vector.memset(ones_f, 1.0)

            # --- phase 1: qkv projection + RoPE --------------------------
            def rope_chunk(psum, dest, h, tcn):
                tmp = work.tile([128, 512], BF, tag="rope_tmp", bufs=3, name="tmp")
                nc.scalar.copy(out=tmp, in_=psum)
                sw = work.tile([128, 512], BF, tag="rope_sw", bufs=3, name="sw")
                nc.sync.dma_start(out=sw[0:64, :], in_=tmp[64:128, :])
                nc.sync.dma_start(out=sw[64:128, :], in_=tmp[0:64, :])
                t1 = work.tile([128, 512], BF, tag="rope_t1", bufs=2, name="t1")
                nc.vector.tensor_mul(t1, tmp, cosf_sb[:, tcn * 512:(tcn + 1) * 512])
                t2 = work.tile([128, 512], BF, tag="rope_t2", bufs=2, name="t2")
                nc.vector.tensor_mul(t2, sw, sins_sb[:, tcn * 512:(tcn + 1) * 512])
                nc.vector.tensor_add(dest[:, h, tcn * 512:(tcn + 1) * 512], t1, t2)

            with tc.tile_pool(name="ps1", bufs=8, space="PSUM") as ps1:
                for tcn in range(n_tc):
                    pq = [ps1.tile([128, 512], F32, tag="p1", name=f"pq{tcn}_{i}")
                          for i in range(HPC)]
                    pk = [ps1.tile([128, 512], F32, tag="p1", name=f"pk{tcn}_{i}")
                          for i in range(HPC)]
                    for ct in range(n_ct):
                        xch = work.tile([128, 512], BF, tag="xchunk", bufs=3, name="xch")
                        nc.sync.dma_start(
                            out=xch,
                            in_=xT[ct * 128:(ct + 1) * 128, tcn * 512:(tcn + 1) * 512])
                        for h in range(HPC):
                            nc.tensor.matmul(
                                pq[h], wq_sb[:, ct, h * D:(h + 1) * D], xch,
                                start=(ct == 0), stop=(ct == n_ct - 1))
                            nc.tensor.matmul(
                                pk[h], wk_sb[:, ct, h * D:(h + 1) * D], xch,
                                start=(ct == 0), stop=(ct == n_ct - 1))
                    for h in range(HPC):
                        rope_chunk(pq[h], qrot, h, tcn)
                        rope_chunk(pk[h], krot, h, tcn)

                # 1b: v in natural [t, d] layout (re-streams xT)
                for tcn in range(n_tc):
                    pv = [ps1.tile([128, DV], F32, tag="p1", name=f"pv{tcn}_{i}")
                          for i in range(4)]
                    for ct in range(n_ct):
                        xch = work.tile([128, 512], BF, tag="xchunk", bufs=3, name="xch")
                        nc.sync.dma_start(
                            out=xch,
                            in_=xT[ct * 128:(ct + 1) * 128, tcn * 512:(tcn + 1) * 512])
                        for ts in range(4):
                            nc.tensor.matmul(
                                pv[ts], xch[:, ts * 128:(ts + 1) * 128], wv_sb[:, ct, :],
                                start=(ct == 0), stop=(ct == n_ct - 1))
                    for ts in range(4):
                        nc.scalar.copy(out=v_sb[:, tcn * 4 + ts, :], in_=pv[ts])

            # --- phase 2+3: attention + projection -----------------------
            with tc.tile_pool(name="ps2", bufs=1, space="PSUM") as ps2:
                for qc in range(n_qc):
                    for h in range(HPC):
                        nkt = 4 * qc + 4
                        po = ps2.tile([128, 512], F32, tag="o", bufs=2, name=f"po{qc}_{h}")
                        pz = ps2.tile([1, 512], F32, tag="z", bufs=1, name=f"pz{qc}_{h}")
                        for kt in range(nkt):
                            r = kt - 4 * qc
                            off = 128 * r if r >= 0 else 0
                            N = 512 - off
                            ps_s = ps2.tile([128, 512], F32, tag="s", bufs=3,
                                            name=f"s{qc}_{h}_{kt}")
                            nc.tensor.matmul(
                                ps_s[:, :N],
                                krot[:, h, kt * 128:(kt + 1) * 128],
                                qrot[:, h, qc * 512 + off:(qc + 1) * 512],
                                start=True, stop=True)
                            expT = work.tile([128, 512], BF, tag="expT", bufs=4,
                                             name="expT")
                            nc.scalar.activation(
                                out=expT[:, :N], in_=ps_s[:, :N],
                                func=mybir.ActivationFunctionType.Exp,
                                scale=SCALE)
                            if r >= 0:
                                nc.vector.tensor_mul(
                                    expT[:, 0:128], expT[:, 0:128], tri_sb)
                            nc.tensor.matmul(
                                po[:, off:], v_sb[:, kt, h * D:(h + 1) * D],
                                expT[:, :N],
                                start=(kt == 0), stop=(kt == nkt - 1))
                            nc.tensor.matmul(
                                pz[:, off:], ones_bf, expT[:, :N],
                                start=(kt == 0), stop=(kt == nkt - 1))
                        recipz = work.tile([1, 512], F32, tag="recipz", bufs=2,
                                           name="recipz")
                        nc.vector.reciprocal(out=recipz, in_=pz)
                        pzb = ps2.tile([128, 512], F32, tag="zb", bufs=1,
                                       name=f"pzb{qc}_{h}")
                        nc.tensor.matmul(pzb, ones_f, recipz, start=True, stop=True)
                        zb_sb = work.tile([128, 512], F32, tag="zb_sb", bufs=2,
                                          name="zb_sb")
                        nc.scalar.copy(out=zb_sb, in_=pzb)
                        nc.vector.tensor_mul(
                            attoT[:, h, qc * 512:(qc + 1) * 512], po, zb_sb)

                    for ti in range(qc * 4, qc * 4 + 4):
                        for cc in range(C // 512):
                            py = ps2.tile([128, 512], F32, tag="y", bufs=1,
                                          name=f"py{ti}_{cc}")
                            for h in range(HPC):
                                nc.tensor.matmul(
                                    py, attoT[:, h, ti * 128:(ti + 1) * 128],
                                    wp_sb[:, h, cc * 512:(cc + 1) * 512],
                                    start=(h == 0), stop=(h == HPC - 1))
                            y_sb = work.tile([128, 512], F32, tag="y_sb", bufs=3,
                                             name="y_sb")
                            if (ti + cc) % 2 == 0:
                                nc.scalar.copy(out=y_sb, in_=py)
                            else:
                                nc.vector.tensor_copy(out=y_sb, in_=py)
                            nc.sync.dma_start(
                                out=y[ti * 128:(ti + 1) * 128, cc * 512:(cc + 1) * 512],
                                in_=y_sb)

    nc.compile()
    return nc


_NC_CACHE = None


def get_nc():
    global _NC_CACHE
    if _NC_CACHE is None:
        _NC_CACHE = build_nc()
    return _NC_CACHE


def make_in_maps(x, w_attn, w_proj, start_index):
    """Full inputs -> per-core in_maps (core c: batch c//4, head group c%4)."""
    x = np.asarray(x, dtype=np.float32)
    w_attn = np.asarray(w_attn, dtype=np.float32)
    w_proj = np.asarray(w_proj, dtype=np.float32)
    si = int(np.asarray(start_index).item()) if np.asarray(start_index).shape == () \
        else int(start_index)

    wq_full = w_attn[:, 0 * C:1 * C]
    wk_full = w_attn[:, 1 * C:2 * C]
    wv_full = w_attn[:, 2 * C:3 * C]

    in_maps = []
    for c in range(N_CORES):
        b, g = divmod(c, GROUPS)
        cols = slice(g * DV, (g + 1) * DV)
        in_maps.append(make_core_inputs(
            x[b], wq_full[:, cols], wk_full[:, cols], wv_full[:, cols],
            w_proj[g * DV:(g + 1) * DV, :], si))
    return in_maps


def kernel(x, w_attn, w_proj, start_index):
    nc = get_nc()
    in_maps = make_in_maps(x, w_attn, w_proj, start_index)
    res = run_bass_kernel_spmd(nc, in_maps, core_ids=list(range(N_CORES)))
    out = np.zeros((B, T, C), dtype=np.float32)
    for c in range(N_CORES):
        b = c // GROUPS
        out[b] += res.results[c]["y"]
    return out


# revision 11
# speedup vs baseline: 1.0253x; 1.0253x over previous
"""Causal self-attention with RoPE (B=2, T=2048, C=2048, 16 heads) on 8 TRN2 cores.

Sharding: data-parallel over batch (2) x tensor-parallel over head groups
(16 heads -> 4 groups of 4), Megatron-style. Core c handles batch c//4 and
head group c%4: fused QKV projection (column-parallel slice of w_attn), RoPE,
causal attention for its 4 heads, and the row-parallel c_proj partial product.
No on-device collectives: the c_proj all-reduce is a host-side sum of the 4
partials per batch element.

Per-core device pipeline (all matmuls bf16, fp32 PSUM accumulation):
  - qT/kT produced directly in [d, t] layout (weights as stationary operand,
    host-transposed xT as moving operand); v in natural [t, d] layout.
  - RoPE applied in [d, t] layout. The rotation pair-shuffle is turned into a
    partition half-swap by host-permuting the q/k weight columns
    (even dims first, odd dims second); the swap itself is 2 SBUF-to-SBUF
    DMAs, then 2 muls + 1 add against host-precomputed cos/sin tables.
  - Scores computed transposed ([k, q]) so the exp() output feeds the PV
    matmul directly (no PE transposes anywhere in attention).
  - Lazy softmax: no max subtraction (scores are O(5) here; exp is safe in
    fp32/bf16 range), denominator Z from a ones-vector matmul, and the
    1/Z normalization applied to the small PV output via DVE reciprocal +
    K=1 broadcast matmul.
  - Causal masking: score tiles strictly above the diagonal are never
    computed; diagonal 128x128 subtiles get a 0/1 triangular mask multiply.
"""

import sys

if "/opt/trn_rl_repo" not in sys.path:
    sys.path.insert(0, "/opt/trn_rl_repo")

import numpy as np
import ml_dtypes

import concourse.bacc as bacc
import concourse.tile as tile
from concourse import mybir
from concourse.bass_utils import run_bass_kernel_spmd

BF16 = ml_dtypes.bfloat16
F32 = mybir.dt.float32
BF = mybir.dt.bfloat16

B, T, C = 2, 2048, 2048
N_HEAD = 16
D = 128
N_CORES = 8
GROUPS = 4            # head groups (tensor-parallel)
HPC = N_HEAD // GROUPS  # heads per core = 4
DV = HPC * D          # per-core qkv width = 512
ROPE_THETA = 10000.0


def _deinterleave_perm():
    return np.concatenate([np.arange(0, D, 2), np.arange(1, D, 2)])


def _rope_tables(start_index):
    j = np.arange(D // 2, dtype=np.float64)
    inv_freq = 1.0 / (ROPE_THETA ** (2.0 * j / D))
    pos = np.arange(T, dtype=np.float64) + float(start_index)
    ang = np.outer(inv_freq, pos)  # [64, T]
    cos, sin = np.cos(ang), np.sin(ang)
    cosf = np.concatenate([cos, cos], axis=0).astype(np.float32)
    sins = np.concatenate([-sin, sin], axis=0).astype(np.float32)
    return cosf, sins


def make_core_inputs(x_b, wq_raw, wk_raw, wv_raw, wp_raw, start_index):
    """Host prep for one core: permute/cast weights, build tables."""
    perm = _deinterleave_perm()

    def permute_heads(w):
        w = w.reshape(C, HPC, D)
        return np.ascontiguousarray(w[:, :, perm].reshape(C, HPC * D))

    cosf, sins = _rope_tables(start_index)
    tri = (np.arange(128)[:, None] <= np.arange(128)[None, :]).astype(np.float32)
    return {
        "xT": np.ascontiguousarray(x_b.T).astype(BF16),
        "wq": permute_heads(wq_raw).astype(BF16),
        "wk": permute_heads(wk_raw).astype(BF16),
        "wv": np.ascontiguousarray(wv_raw).astype(BF16),
        "wp": np.ascontiguousarray(wp_raw).astype(BF16),
        "cosf": cosf.astype(BF16),
        "sins": sins.astype(BF16),
        "tri": tri.astype(BF16),
    }


def build_nc(debug=False, SBUFS=3, ZBUFS=1, YBUFS=2):
    """Build + bass-compile the per-core program (same on all 8 cores)."""
    n_tc = T // 512   # 4
    n_ct = C // 128   # 16
    n_kt = T // 128   # 16
    n_qc = T // 512   # 4
    SCALE = 1.0 / float(np.sqrt(D))

    nc = bacc.Bacc(None, target_bir_lowering=False, debug=debug)

    xT = nc.declare_dram_parameter("xT", [C, T], BF, isOutput=False)
    wq = nc.declare_dram_parameter("wq", [C, DV], BF, isOutput=False)
    wk = nc.declare_dram_parameter("wk", [C, DV], BF, isOutput=False)
    wv = nc.declare_dram_parameter("wv", [C, DV], BF, isOutput=False)
    wp = nc.declare_dram_parameter("wp", [DV, C], BF, isOutput=False)
    cosf = nc.declare_dram_parameter("cosf", [128, T], BF, isOutput=False)
    sins = nc.declare_dram_parameter("sins", [128, T], BF, isOutput=False)
    tri = nc.declare_dram_parameter("tri", [128, 128], BF, isOutput=False)
    y = nc.declare_dram_parameter("y", [T, C], F32, isOutput=True)

    def bcast_head(ap, n=HPC):
        """[128, N] AP -> [128, n, N] with a 0-step head dim (free-dim bcast)."""
        import concourse.bass as bass
        return bass.AP(tensor=ap.tensor, offset=ap.offset,
                       ap=[ap.ap[0], [0, n], ap.ap[1]])

    with tile.TileContext(nc) as tc:
        with tc.tile_pool(name="const", bufs=1) as const, \
             tc.tile_pool(name="big", bufs=1) as big, \
             tc.tile_pool(name="work", bufs=3) as work:

            wq_sb = const.tile([128, n_ct, DV], BF)
            wk_sb = const.tile([128, n_ct, DV], BF)
            wv_sb = const.tile([128, n_ct, DV], BF)
            wp_sb = const.tile([128, HPC, C], BF)
            cosf_sb = const.tile([128, T], BF)
            sins_sb = const.tile([128, T], BF)
            tri_sb = const.tile([128, 128], BF)
            ones_bf = const.tile([128, 1], BF)

            qrot = big.tile([128, HPC, T], BF)
            krot = big.tile([128, HPC, T], BF)
            v_sb = big.tile([128, n_kt, DV], BF)
            attoT = big.tile([128, HPC, T], BF)

            # weights/tables: batched DMAs, spread over the scalar HWDGE queue
            # and the gpsimd SWDGE so they never block the x stream (sync queue).
            # quarter-granularity so the first matmuls can start early.
            nq = n_ct // 4
            wsplits = [(0, 4), (4, 8), (8, 12), (12, 16)]
            for lo, hi in wsplits:
                nc.scalar.dma_start(
                    out=wq_sb[:, lo:hi, :],
                    in_=wq[lo * 128:hi * 128, :].rearrange("(a p) f -> p a f", p=128))
                nc.scalar.dma_start(
                    out=wk_sb[:, lo:hi, :],
                    in_=wk[lo * 128:hi * 128, :].rearrange("(a p) f -> p a f", p=128))
            nc.scalar.dma_start(out=cosf_sb, in_=cosf[:, :])
            nc.scalar.dma_start(out=sins_sb, in_=sins[:, :])
            for qtr in range(4):
                rows = slice(qtr * nq * 128, (qtr + 1) * nq * 128)
                sl = slice(qtr * nq, (qtr + 1) * nq)
                nc.gpsimd.dma_start(
                    out=wv_sb[:, sl, :],
                    in_=wv[rows, :].rearrange("(a p) f -> p a f", p=128))
            nc.gpsimd.dma_start(out=tri_sb, in_=tri[:, :])
            nc.gpsimd.dma_start(
                out=wp_sb, in_=wp[:, :].rearrange("(h p) f -> p h f", p=128))
            nc.vector.memset(ones_bf, 1.0)

            # --- phase 1: qkv projection + RoPE --------------------------
            # one pass over xT per t-chunk: q,k (8 psums), then v (4 psums)
            # from the same SBUF-resident x chunk.
            def rope4(psums, dest, tcn):
                """4 head psums [128,512] f32 -> dest[:, :, tc] rotated bf16."""
                tmp4 = work.tile([128, HPC, 512], BF, tag="rope_tmp", bufs=2,
                                 name="tmp4")
                for h in range(HPC):
                    nc.scalar.copy(out=tmp4[:, h, :], in_=psums[h])
                sw4 = work.tile([128, HPC, 512], BF, tag="rope_sw", bufs=2,
                                name="sw4")
                nc.sync.dma_start(out=sw4[0:64, :, :], in_=tmp4[64:128, :, :])
                nc.sync.dma_start(out=sw4[64:128, :, :], in_=tmp4[0:64, :, :])
                cosb = bcast_head(cosf_sb[:, tcn * 512:(tcn + 1) * 512])
                sinb = bcast_head(sins_sb[:, tcn * 512:(tcn + 1) * 512])
                nc.vector.tensor_mul(tmp4, tmp4, cosb)
                nc.vector.tensor_mul(sw4, sw4, sinb)
                nc.vector.tensor_add(
                    dest[:, :, tcn * 512:(tcn + 1) * 512], tmp4, sw4)

            with tc.tile_pool(name="ps1", bufs=8, space="PSUM") as ps1:
                nh = n_ct // 2
                for tcn in range(n_tc):
                    xtc_halves = []
                    nsplit = 2
                    for half in range(2):
                        xh = work.tile([128, nh, 512], BF, tag="xtc", bufs=3,
                                       name=f"xtc{tcn}_{half}")
                        step = nh // nsplit
                        for qtr in range(nsplit):
                            a0 = half * nh + qtr * step
                            rows = slice(a0 * 128, (a0 + step) * 128)
                            sl = slice(qtr * step, (qtr + 1) * step)
                            nc.sync.dma_start(
                                out=xh[:, sl, :],
                                in_=xT[rows, tcn * 512:(tcn + 1) * 512]
                                .rearrange("(a p) t -> p a t", p=128))
                        xtc_halves.append(xh)
                    def xct(ct):
                        return xtc_halves[ct // nh][:, ct % nh, :]
                    pq = [ps1.tile([128, 512], F32, tag="p1", name=f"pq{tcn}_{i}")
                          for i in range(HPC)]
                    pk = [ps1.tile([128, 512], F32, tag="p1", name=f"pk{tcn}_{i}")
                          for i in range(HPC)]
                    for ct in range(n_ct):
                        for h in range(HPC):
                            nc.tensor.matmul(
                                pq[h], wq_sb[:, ct, h * D:(h + 1) * D], xct(ct),
                                start=(ct == 0), stop=(ct == n_ct - 1))
                            nc.tensor.matmul(
                                pk[h], wk_sb[:, ct, h * D:(h + 1) * D], xct(ct),
                                start=(ct == 0), stop=(ct == n_ct - 1))
                    rope4(pq, qrot, tcn)
                    rope4(pk, krot, tcn)

                    # v for this t-chunk, from the same resident x chunk
                    pv = [ps1.tile([128, DV], F32, tag="p1", name=f"pv{tcn}_{i}")
                          for i in range(4)]
                    for ct in range(n_ct):
                        for ts in range(4):
                            nc.tensor.matmul(
                                pv[ts], xct(ct)[:, ts * 128:(ts + 1) * 128],
                                wv_sb[:, ct, :],
                                start=(ct == 0), stop=(ct == n_ct - 1))
                    for ts in range(4):
                        nc.scalar.copy(out=v_sb[:, tcn * 4 + ts, :], in_=pv[ts])

            # --- phase 2+3: attention + projection -----------------------
            import concourse.bass as bass
            with tc.tile_pool(name="dram", bufs=2, space="DRAM") as dram, \
                 tc.tile_pool(name="ps2", bufs=1, space="PSUM") as ps2:
                def proj_block(qc):
                    for ti in range(qc * 4, qc * 4 + 4):
                        y_sb = work.tile([128, C], F32, tag="y_sb", bufs=2,
                                         name="y_sb")
                        for cc in range(C // 512):
                            py = ps2.tile([128, 512], F32, tag="y", bufs=YBUFS,
                                          name=f"py{ti}_{cc}")
                            for h in range(HPC):
                                nc.tensor.matmul(
                                    py, attoT[:, h, ti * 128:(ti + 1) * 128],
                                    wp_sb[:, h, cc * 512:(cc + 1) * 512],
                                    start=(h == 0), stop=(h == HPC - 1))
                            if cc % 2 == 0:
                                nc.scalar.copy(out=y_sb[:, cc * 512:(cc + 1) * 512], in_=py)
                            else:
                                nc.vector.tensor_copy(out=y_sb[:, cc * 512:(cc + 1) * 512], in_=py)
                        nc.scalar.dma_start(
                            out=y[ti * 128:(ti + 1) * 128, :], in_=y_sb)

                for qc in range(n_qc):
                    for h in range(HPC):
                        nkt = 4 * qc + 4
                        po = ps2.tile([128, 512], F32, tag="o", bufs=2, name=f"po{qc}_{h}")
                        pz = ps2.tile([1, 512], F32, tag="z", bufs=ZBUFS, name=f"pz{qc}_{h}")
                        for kt in range(nkt):
                            r = kt - 4 * qc
                            off = 128 * r if r >= 0 else 0
                            N = 512 - off
                            ps_s = ps2.tile([128, 512], F32, tag="s", bufs=SBUFS,
                                            name=f"s{qc}_{h}_{kt}")
                            nc.tensor.matmul(
                                ps_s[:, :N],
                                krot[:, h, kt * 128:(kt + 1) * 128],
                                qrot[:, h, qc * 512 + off:(qc + 1) * 512],
                                start=True, stop=True)
                            expT = work.tile([128, 512], BF, tag="expT", bufs=4,
                                             name="expT")
                            nc.scalar.activation(
                                out=expT[:, :N], in_=ps_s[:, :N],
                                func=mybir.ActivationFunctionType.Exp,
                                scale=SCALE)
                            if r >= 0:
                                nc.vector.tensor_mul(
                                    expT[:, 0:128], expT[:, 0:128], tri_sb)
                            nc.tensor.matmul(
                                po[:, off:], v_sb[:, kt, h * D:(h + 1) * D],
                                expT[:, :N],
                                start=(kt == 0), stop=(kt == nkt - 1))
                            nc.tensor.matmul(
                                pz[:, off:], ones_bf, expT[:, :N],
                                start=(kt == 0), stop=(kt == nkt - 1))
                        recipz = work.tile([1, 512], F32, tag="recipz", bufs=2,
                                           name="recipz")
                        nc.vector.reciprocal_approx_fast(out=recipz, in_=pz)
                        # broadcast 1/Z to all partitions via a DRAM bounce on
                        # the (idle in this phase) sync queue: SBUF->DRAM 2KB,
                        # then a partition-broadcast DRAM->SBUF read.
                        zdram = dram.tile([1, 512], F32, tag="zd", bufs=2,
                                          name=f"zd{qc}_{h}")
                        nc.sync.dma_start(out=zdram, in_=recipz)
                        zb_sb = work.tile([128, 512], F32, tag="zb_sb", bufs=2,
                                          name="zb_sb")
                        zb_bc = bass.AP(tensor=zdram.tensor, offset=zdram.offset,
                                        ap=[[0, 128]] + list(zdram.ap[1:]))
                        nc.sync.dma_start(out=zb_sb, in_=zb_bc)
                        nc.vector.tensor_mul(
                            attoT[:, h, qc * 512:(qc + 1) * 512], po, zb_sb)

                    if qc > 0:
                        proj_block(qc - 1)
                proj_block(n_qc - 1)

    nc.compile()
    return nc


_NC_CACHE = None


def get_nc():
    global _NC_CACHE
    if _NC_CACHE is None:
        _NC_CACHE = build_nc()
    return _NC_CACHE


def make_in_maps(x, w_attn, w_proj, start_index):
    """Full inputs -> per-core in_maps (core c: batch c//4, head group c%4)."""
    x = np.asarray(x, dtype=np.float32)
    w_attn = np.asarray(w_attn, dtype=np.float32)
    w_proj = np.asarray(w_proj, dtype=np.float32)
    si = int(np.asarray(start_index).item()) if np.asarray(start_index).shape == () \
        else int(start_index)

    wq_full = w_attn[:, 0 * C:1 * C]
    wk_full = w_attn[:, 1 * C:2 * C]
    wv_full = w_attn[:, 2 * C:3 * C]

    in_maps = []
    for c in range(N_CORES):
        b, g = divmod(c, GROUPS)
        cols = slice(g * DV, (g + 1) * DV)
        in_maps.append(make_core_inputs(
            x[b], wq_full[:, cols], wk_full[:, cols], wv_full[:, cols],
            w_proj[g * DV:(g + 1) * DV, :], si))
    return in_maps


def kernel(x, w_attn, w_proj, start_index):
    nc = get_nc()
    in_maps = make_in_maps(x, w_attn, w_proj, start_index)
    res = run_bass_kernel_spmd(nc, in_maps, core_ids=list(range(N_CORES)))
    out = np.zeros((B, T, C), dtype=np.float32)
    for c in range(N_CORES):
        b = c // GROUPS
        out[b] += res.results[c]["y"]
    return out


# revision 12
# speedup vs baseline: 1.2490x; 1.2181x over previous
"""Causal self-attention with RoPE (B=2, T=2048, C=2048, 16 heads) on 8 TRN2 cores.

Sharding: data-parallel over batch (2) x tensor-parallel over head groups
(16 heads -> 4 groups of 4), Megatron-style. Core c handles batch c//4 and
head group c%4: fused QKV projection (column-parallel slice of w_attn), RoPE,
causal attention for its 4 heads, and the row-parallel c_proj partial product.
No on-device collectives: the c_proj all-reduce is a host-side sum of the 4
partials per batch element.

Per-core device pipeline (all matmuls bf16, fp32 PSUM accumulation):
  - qT/kT produced directly in [d, t] layout (weights as stationary operand,
    host-transposed xT as moving operand); v in natural [t, d] layout.
  - RoPE applied in [d, t] layout. The rotation pair-shuffle is turned into a
    partition half-swap by host-permuting the q/k weight columns
    (even dims first, odd dims second); the swap itself is 2 SBUF-to-SBUF
    DMAs, then 2 muls + 1 add against host-precomputed cos/sin tables.
  - Scores computed transposed ([k, q]) so the exp() output feeds the PV
    matmul directly (no PE transposes anywhere in attention).
  - Lazy softmax: no max subtraction (scores are O(5) here; exp is safe in
    fp32/bf16 range), denominator Z from a ones-vector matmul, and the
    1/Z normalization applied to the small PV output via DVE reciprocal +
    K=1 broadcast matmul.
  - Causal masking: score tiles strictly above the diagonal are never
    computed; diagonal 128x128 subtiles get a 0/1 triangular mask multiply.
"""

import sys

if "/opt/trn_rl_repo" not in sys.path:
    sys.path.insert(0, "/opt/trn_rl_repo")

import numpy as np
import ml_dtypes

import concourse.bacc as bacc
import concourse.tile as tile
from concourse import mybir
from concourse.bass_utils import run_bass_kernel_spmd

BF16 = ml_dtypes.bfloat16
F32 = mybir.dt.float32
BF = mybir.dt.bfloat16

B, T, C = 2, 2048, 2048
N_HEAD = 16
D = 128
N_CORES = 8
GROUPS = 4            # head groups (tensor-parallel)
HPC = N_HEAD // GROUPS  # heads per core = 4
DV = HPC * D          # per-core qkv width = 512
ROPE_THETA = 10000.0


def _deinterleave_perm():
    return np.concatenate([np.arange(0, D, 2), np.arange(1, D, 2)])


def _rope_tables(start_index):
    j = np.arange(D // 2, dtype=np.float64)
    inv_freq = 1.0 / (ROPE_THETA ** (2.0 * j / D))
    pos = np.arange(T, dtype=np.float64) + float(start_index)
    ang = np.outer(inv_freq, pos)  # [64, T]
    cos, sin = np.cos(ang), np.sin(ang)
    cosf = np.concatenate([cos, cos], axis=0).astype(np.float32)
    sins = np.concatenate([-sin, sin], axis=0).astype(np.float32)
    return cosf, sins


def make_core_inputs(x_b, wq_raw, wk_raw, wv_raw, wp_raw, start_index):
    """Host prep for one core: permute/cast weights, build tables."""
    perm = _deinterleave_perm()

    def permute_heads(w):
        w = w.reshape(C, HPC, D)
        return np.ascontiguousarray(w[:, :, perm].reshape(C, HPC * D))

    cosf, sins = _rope_tables(start_index)
    tri = (np.arange(128)[:, None] <= np.arange(128)[None, :]).astype(np.float32)
    return {
        "xT": np.ascontiguousarray(x_b.T).astype(BF16),
        "wq": permute_heads(wq_raw).astype(BF16),
        "wk": permute_heads(wk_raw).astype(BF16),
        "wv": np.ascontiguousarray(wv_raw).astype(BF16),
        "wp": np.ascontiguousarray(wp_raw).astype(BF16),
        "cosf": cosf.astype(BF16),
        "sins": sins.astype(BF16),
        "tri": tri.astype(BF16),
    }


def build_nc(debug=False, SBUFS=3, ZBUFS=1, YBUFS=2):
    """Build + bass-compile the per-core program (same on all 8 cores)."""
    n_tc = T // 512   # 4
    n_ct = C // 128   # 16
    n_kt = T // 128   # 16
    n_qc = T // 512   # 4
    SCALE = 1.0 / float(np.sqrt(D))

    nc = bacc.Bacc(None, target_bir_lowering=False, debug=debug)

    xT = nc.declare_dram_parameter("xT", [C, T], BF, isOutput=False)
    wq = nc.declare_dram_parameter("wq", [C, DV], BF, isOutput=False)
    wk = nc.declare_dram_parameter("wk", [C, DV], BF, isOutput=False)
    wv = nc.declare_dram_parameter("wv", [C, DV], BF, isOutput=False)
    wp = nc.declare_dram_parameter("wp", [DV, C], BF, isOutput=False)
    cosf = nc.declare_dram_parameter("cosf", [128, T], BF, isOutput=False)
    sins = nc.declare_dram_parameter("sins", [128, T], BF, isOutput=False)
    tri = nc.declare_dram_parameter("tri", [128, 128], BF, isOutput=False)
    y = nc.declare_dram_parameter("y", [T, C], F32, isOutput=True)

    def bcast_head(ap, n=HPC):
        """[128, N] AP -> [128, n, N] with a 0-step head dim (free-dim bcast)."""
        import concourse.bass as bass
        return bass.AP(tensor=ap.tensor, offset=ap.offset,
                       ap=[ap.ap[0], [0, n], ap.ap[1]])

    with tile.TileContext(nc) as tc:
        with tc.tile_pool(name="const", bufs=1) as const, \
             tc.tile_pool(name="big", bufs=1) as big, \
             tc.tile_pool(name="work", bufs=3) as work:

            wq_sb = const.tile([128, n_ct, DV], BF)
            wk_sb = const.tile([128, n_ct, DV], BF)
            wv_sb = const.tile([128, n_ct, DV], BF)
            wp_sb = const.tile([128, HPC, C], BF)
            cosf_sb = const.tile([128, T], BF)
            sins_sb = const.tile([128, T], BF)
            tri_sb = const.tile([128, 128], BF)
            ones_bf = const.tile([128, 1], BF)

            qrot = big.tile([128, HPC, T], BF)
            krot = big.tile([128, HPC, T], BF)
            v_sb = big.tile([128, n_kt, DV], BF)
            attoT = big.tile([128, HPC, T], BF)

            # weights/tables: batched DMAs, spread over the scalar HWDGE queue
            # and the gpsimd SWDGE so they never block the x stream (sync queue).
            # quarter-granularity so the first matmuls can start early.
            nq = n_ct // 4
            wsplits = [(0, 4), (4, 8), (8, 12), (12, 16)]
            for lo, hi in wsplits:
                nc.scalar.dma_start(
                    out=wq_sb[:, lo:hi, :],
                    in_=wq[lo * 128:hi * 128, :].rearrange("(a p) f -> p a f", p=128))
                nc.scalar.dma_start(
                    out=wk_sb[:, lo:hi, :],
                    in_=wk[lo * 128:hi * 128, :].rearrange("(a p) f -> p a f", p=128))
            nc.scalar.dma_start(out=cosf_sb, in_=cosf[:, :])
            nc.scalar.dma_start(out=sins_sb, in_=sins[:, :])
            for qtr in range(4):
                rows = slice(qtr * nq * 128, (qtr + 1) * nq * 128)
                sl = slice(qtr * nq, (qtr + 1) * nq)
                nc.gpsimd.dma_start(
                    out=wv_sb[:, sl, :],
                    in_=wv[rows, :].rearrange("(a p) f -> p a f", p=128))
            nc.gpsimd.dma_start(out=tri_sb, in_=tri[:, :])
            nc.gpsimd.dma_start(
                out=wp_sb, in_=wp[:, :].rearrange("(h p) f -> p h f", p=128))
            nc.vector.memset(ones_bf, 1.0)

            # --- phase 1: qkv projection + RoPE --------------------------
            # one pass over xT per t-chunk: q,k (8 psums), then v (4 psums)
            # from the same SBUF-resident x chunk.
            def rope4(psums, dest, tcn):
                """4 head psums [128,512] f32 -> dest[:, :, tc] rotated bf16."""
                tmp4 = work.tile([128, HPC, 512], BF, tag="rope_tmp", bufs=2,
                                 name="tmp4")
                for h in range(HPC):
                    nc.scalar.copy(out=tmp4[:, h, :], in_=psums[h])
                sw4 = work.tile([128, HPC, 512], BF, tag="rope_sw", bufs=2,
                                name="sw4")
                nc.scalar.dma_start(out=sw4[0:64, :, :], in_=tmp4[64:128, :, :])
                nc.scalar.dma_start(out=sw4[64:128, :, :], in_=tmp4[0:64, :, :])
                cosb = bcast_head(cosf_sb[:, tcn * 512:(tcn + 1) * 512])
                sinb = bcast_head(sins_sb[:, tcn * 512:(tcn + 1) * 512])
                nc.vector.tensor_mul(tmp4, tmp4, cosb)
                nc.vector.tensor_mul(sw4, sw4, sinb)
                nc.vector.tensor_add(
                    dest[:, :, tcn * 512:(tcn + 1) * 512], tmp4, sw4)

            with tc.tile_pool(name="ps1", bufs=8, space="PSUM") as ps1:
                nh = n_ct // 2
                for tcn in range(n_tc):
                    xtc_halves = []
                    nsplit = 2
                    for half in range(2):
                        xh = work.tile([128, nh, 512], BF, tag="xtc", bufs=3,
                                       name=f"xtc{tcn}_{half}")
                        step = nh // nsplit
                        for qtr in range(nsplit):
                            a0 = half * nh + qtr * step
                            rows = slice(a0 * 128, (a0 + step) * 128)
                            sl = slice(qtr * step, (qtr + 1) * step)
                            nc.sync.dma_start(
                                out=xh[:, sl, :],
                                in_=xT[rows, tcn * 512:(tcn + 1) * 512]
                                .rearrange("(a p) t -> p a t", p=128))
                        xtc_halves.append(xh)
                    def xct(ct):
                        return xtc_halves[ct // nh][:, ct % nh, :]
                    pq = [ps1.tile([128, 512], F32, tag="p1", name=f"pq{tcn}_{i}")
                          for i in range(HPC)]
                    pk = [ps1.tile([128, 512], F32, tag="p1", name=f"pk{tcn}_{i}")
                          for i in range(HPC)]
                    for ct in range(n_ct):
                        for h in range(HPC):
                            nc.tensor.matmul(
                                pq[h], wq_sb[:, ct, h * D:(h + 1) * D], xct(ct),
                                start=(ct == 0), stop=(ct == n_ct - 1))
                            nc.tensor.matmul(
                                pk[h], wk_sb[:, ct, h * D:(h + 1) * D], xct(ct),
                                start=(ct == 0), stop=(ct == n_ct - 1))
                    rope4(pq, qrot, tcn)
                    rope4(pk, krot, tcn)

                    # v for this t-chunk, from the same resident x chunk
                    pv = [ps1.tile([128, DV], F32, tag="p1", name=f"pv{tcn}_{i}")
                          for i in range(4)]
                    for ct in range(n_ct):
                        for ts in range(4):
                            nc.tensor.matmul(
                                pv[ts], xct(ct)[:, ts * 128:(ts + 1) * 128],
                                wv_sb[:, ct, :],
                                start=(ct == 0), stop=(ct == n_ct - 1))
                    for ts in range(4):
                        nc.scalar.copy(out=v_sb[:, tcn * 4 + ts, :], in_=pv[ts])

            # --- phase 2+3: attention + projection -----------------------
            import concourse.bass as bass
            with tc.tile_pool(name="dram", bufs=2, space="DRAM") as dram, \
                 tc.tile_pool(name="ps2", bufs=1, space="PSUM") as ps2:
                def proj_block(qc):
                    for ti in range(qc * 4, qc * 4 + 4):
                        y_sb = work.tile([128, C], F32, tag="y_sb", bufs=2,
                                         name="y_sb")
                        for cc in range(C // 512):
                            py = ps2.tile([128, 512], F32, tag="y", bufs=YBUFS,
                                          name=f"py{ti}_{cc}")
                            for h in range(HPC):
                                nc.tensor.matmul(
                                    py, attoT[:, h, ti * 128:(ti + 1) * 128],
                                    wp_sb[:, h, cc * 512:(cc + 1) * 512],
                                    start=(h == 0), stop=(h == HPC - 1))
                            if cc % 2 == 0:
                                nc.scalar.copy(out=y_sb[:, cc * 512:(cc + 1) * 512], in_=py)
                            else:
                                nc.vector.tensor_copy(out=y_sb[:, cc * 512:(cc + 1) * 512], in_=py)
                        nc.scalar.dma_start(
                            out=y[ti * 128:(ti + 1) * 128, :], in_=y_sb)

                for qc in range(n_qc):
                    for h in range(HPC):
                        nkt = 4 * qc + 4
                        po = ps2.tile([128, 512], F32, tag="o", bufs=2, name=f"po{qc}_{h}")
                        pz = ps2.tile([1, 512], F32, tag="z", bufs=ZBUFS, name=f"pz{qc}_{h}")
                        for kt in range(nkt):
                            r = kt - 4 * qc
                            off = 128 * r if r >= 0 else 0
                            N = 512 - off
                            ps_s = ps2.tile([128, 512], F32, tag="s", bufs=SBUFS,
                                            name=f"s{qc}_{h}_{kt}")
                            nc.tensor.matmul(
                                ps_s[:, :N],
                                krot[:, h, kt * 128:(kt + 1) * 128],
                                qrot[:, h, qc * 512 + off:(qc + 1) * 512],
                                start=True, stop=True)
                            expT = work.tile([128, 512], BF, tag="expT", bufs=4,
                                             name="expT")
                            nc.scalar.activation(
                                out=expT[:, :N], in_=ps_s[:, :N],
                                func=mybir.ActivationFunctionType.Exp,
                                scale=SCALE)
                            if r >= 0:
                                nc.vector.tensor_mul(
                                    expT[:, 0:128], expT[:, 0:128], tri_sb)
                            nc.tensor.matmul(
                                po[:, off:], v_sb[:, kt, h * D:(h + 1) * D],
                                expT[:, :N],
                                start=(kt == 0), stop=(kt == nkt - 1))
                            nc.tensor.matmul(
                                pz[:, off:], ones_bf, expT[:, :N],
                                start=(kt == 0), stop=(kt == nkt - 1))
                        recipz = work.tile([1, 512], F32, tag="recipz", bufs=2,
                                           name="recipz")
                        nc.vector.reciprocal_approx_fast(out=recipz, in_=pz)
                        # broadcast 1/Z to all partitions via a DRAM bounce on
                        # the (idle in this phase) sync queue: SBUF->DRAM 2KB,
                        # then a partition-broadcast DRAM->SBUF read.
                        zdram = dram.tile([1, 512], F32, tag="zd", bufs=2,
                                          name=f"zd{qc}_{h}")
                        nc.sync.dma_start(out=zdram, in_=recipz)
                        zb_sb = work.tile([128, 512], F32, tag="zb_sb", bufs=2,
                                          name="zb_sb")
                        zb_bc = bass.AP(tensor=zdram.tensor, offset=zdram.offset,
                                        ap=[[0, 128]] + list(zdram.ap[1:]))
                        nc.sync.dma_start(out=zb_sb, in_=zb_bc)
                        nc.vector.tensor_mul(
                            attoT[:, h, qc * 512:(qc + 1) * 512], po, zb_sb)

                    if qc > 0:
                        proj_block(qc - 1)
                proj_block(n_qc - 1)

    nc.compile()
    return nc


_NC_CACHE = None


def get_nc():
    global _NC_CACHE
    if _NC_CACHE is None:
        _NC_CACHE = build_nc()
    return _NC_CACHE


def make_in_maps(x, w_attn, w_proj, start_index):
    """Full inputs -> per-core in_maps (core c: batch c//4, head group c%4)."""
    x = np.asarray(x, dtype=np.float32)
    w_attn = np.asarray(w_attn, dtype=np.float32)
    w_proj = np.asarray(w_proj, dtype=np.float32)
    si = int(np.asarray(start_index).item()) if np.asarray(start_index).shape == () \
        else int(start_index)

    wq_full = w_attn[:, 0 * C:1 * C]
    wk_full = w_attn[:, 1 * C:2 * C]
    wv_full = w_attn[:, 2 * C:3 * C]

    in_maps = []
    for c in range(N_CORES):
        b, g = divmod(c, GROUPS)
        cols = slice(g * DV, (g + 1) * DV)
        in_maps.append(make_core_inputs(
            x[b], wq_full[:, cols], wk_full[:, cols], wv_full[:, cols],
            w_proj[g * DV:(g + 1) * DV, :], si))
    return in_maps


def kernel(x, w_attn, w_proj, start_index):
    nc = get_nc()
    in_maps = make_in_maps(x, w_attn, w_proj, start_index)
    res = run_bass_kernel_spmd(nc, in_maps, core_ids=list(range(N_CORES)))
    out = np.zeros((B, T, C), dtype=np.float32)
    for c in range(N_CORES):
        b = c // GROUPS
        out[b] += res.results[c]["y"]
    return out


# revision 14
# speedup vs baseline: 3.1597x; 2.5299x over previous
"""Causal self-attention with RoPE (B=2, T=2048, C=2048, 16 heads) on 8 TRN2 cores.

Sharding: data-parallel over batch (2) x tensor-parallel over head groups
(16 heads -> 4 groups of 4), Megatron-style. Core c handles batch c//4 and
head group c%4: fused QKV projection (column-parallel slice of w_attn), RoPE,
causal attention for its 4 heads, and the row-parallel c_proj partial product.
No on-device collectives: the c_proj all-reduce is a host-side sum of the 4
partials per batch element.

Per-core device pipeline (all matmuls bf16, fp32 PSUM accumulation):
  - qT/kT produced directly in [d, t] layout (weights as stationary operand,
    host-transposed xT as moving operand); v in natural [t, d] layout.
  - RoPE applied in [d, t] layout. The rotation pair-shuffle is turned into a
    partition half-swap by host-permuting the q/k weight columns
    (even dims first, odd dims second); the swap itself is 2 SBUF-to-SBUF
    DMAs, then 2 muls + 1 add against host-precomputed cos/sin tables.
  - Scores computed transposed ([k, q]) so the exp() output feeds the PV
    matmul directly (no PE transposes anywhere in attention).
  - Lazy softmax: no max subtraction (scores are O(5) here; exp is safe in
    fp32/bf16 range), denominator Z from a ones-vector matmul, and the
    1/Z normalization applied to the small PV output via DVE reciprocal +
    K=1 broadcast matmul.
  - Causal masking: score tiles strictly above the diagonal are never
    computed; diagonal 128x128 subtiles get a 0/1 triangular mask multiply.
"""

import sys

if "/opt/trn_rl_repo" not in sys.path:
    sys.path.insert(0, "/opt/trn_rl_repo")

import numpy as np
import ml_dtypes

import concourse.bacc as bacc
import concourse.tile as tile
from concourse import mybir
from concourse.bass_utils import run_bass_kernel_spmd

BF16 = ml_dtypes.bfloat16
F32 = mybir.dt.float32
BF = mybir.dt.bfloat16

B, T, C = 2, 2048, 2048
N_HEAD = 16
D = 128
N_CORES = 8
GROUPS = 4            # head groups (tensor-parallel)
HPC = N_HEAD // GROUPS  # heads per core = 4
DV = HPC * D          # per-core qkv width = 512
ROPE_THETA = 10000.0


def _deinterleave_perm():
    return np.concatenate([np.arange(0, D, 2), np.arange(1, D, 2)])


def _rope_tables(start_index):
    j = np.arange(D // 2, dtype=np.float64)
    inv_freq = 1.0 / (ROPE_THETA ** (2.0 * j / D))
    pos = np.arange(T, dtype=np.float64) + float(start_index)
    ang = np.outer(inv_freq, pos)  # [64, T]
    cos, sin = np.cos(ang), np.sin(ang)
    cosf = np.concatenate([cos, cos], axis=0).astype(np.float32)
    sins = np.concatenate([-sin, sin], axis=0).astype(np.float32)
    return cosf, sins


def make_core_inputs(x_b, wq_raw, wk_raw, wv_raw, wp_raw, start_index):
    """Host prep for one core: permute/cast weights, build tables."""
    perm = _deinterleave_perm()

    def permute_heads(w):
        w = w.reshape(C, HPC, D)
        return np.ascontiguousarray(w[:, :, perm].reshape(C, HPC * D))

    cosf, sins = _rope_tables(start_index)
    tri = (np.arange(128)[:, None] <= np.arange(128)[None, :]).astype(np.float32)
    return {
        "xT": np.ascontiguousarray(x_b.T).astype(BF16),
        "wq": permute_heads(wq_raw).astype(BF16),
        "wk": permute_heads(wk_raw).astype(BF16),
        "wv": np.ascontiguousarray(wv_raw).astype(BF16),
        "wp": np.ascontiguousarray(wp_raw).astype(BF16),
        "cosf": cosf.astype(BF16),
        "sins": sins.astype(BF16),
        "tri": tri.astype(BF16),
    }


def build_nc(debug=False, SBUFS=3, ZBUFS=1, YBUFS=2):
    """Build + bass-compile the per-core program (same on all 8 cores)."""
    n_tc = T // 512   # 4
    n_ct = C // 128   # 16
    n_kt = T // 128   # 16
    n_qc = T // 512   # 4
    SCALE = 1.0 / float(np.sqrt(D))

    nc = bacc.Bacc(None, target_bir_lowering=False, debug=debug)

    xT = nc.declare_dram_parameter("xT", [C, T], BF, isOutput=False)
    wq = nc.declare_dram_parameter("wq", [C, DV], BF, isOutput=False)
    wk = nc.declare_dram_parameter("wk", [C, DV], BF, isOutput=False)
    wv = nc.declare_dram_parameter("wv", [C, DV], BF, isOutput=False)
    wp = nc.declare_dram_parameter("wp", [DV, C], BF, isOutput=False)
    cosf = nc.declare_dram_parameter("cosf", [128, T], BF, isOutput=False)
    sins = nc.declare_dram_parameter("sins", [128, T], BF, isOutput=False)
    tri = nc.declare_dram_parameter("tri", [128, 128], BF, isOutput=False)
    y = nc.declare_dram_parameter("y", [T, C], F32, isOutput=True)

    def bcast_head(ap, n=HPC):
        """[128, N] AP -> [128, n, N] with a 0-step head dim (free-dim bcast)."""
        import concourse.bass as bass
        return bass.AP(tensor=ap.tensor, offset=ap.offset,
                       ap=[ap.ap[0], [0, n], ap.ap[1]])

    with tile.TileContext(nc) as tc:
        with tc.tile_pool(name="const", bufs=1) as const, \
             tc.tile_pool(name="big", bufs=1) as big, \
             tc.tile_pool(name="work", bufs=3) as work:

            wq_sb = const.tile([128, n_ct, DV], BF)
            wk_sb = const.tile([128, n_ct, DV], BF)
            wv_sb = const.tile([128, n_ct, DV], BF)
            wp_sb = const.tile([128, HPC, C], BF)
            cosf_sb = const.tile([128, T], BF)
            sins_sb = const.tile([128, T], BF)
            tri_sb = const.tile([128, 128], BF)
            ones_bf = const.tile([128, 1], BF)

            qrot = big.tile([128, HPC, T], BF)
            krot = big.tile([128, HPC, T], BF)
            v_sb = big.tile([128, n_kt, DV], BF)
            attoT = big.tile([128, HPC, T], BF)

            # weights/tables: batched DMAs, spread over the scalar HWDGE queue
            # and the gpsimd SWDGE so they never block the x stream (sync queue).
            # quarter-granularity so the first matmuls can start early.
            nq = n_ct // 4
            wsplits = [(0, 4), (4, 8), (8, 12), (12, 16)]
            for lo, hi in wsplits:
                nc.scalar.dma_start(
                    out=wq_sb[:, lo:hi, :],
                    in_=wq[lo * 128:hi * 128, :].rearrange("(a p) f -> p a f", p=128))
                nc.scalar.dma_start(
                    out=wk_sb[:, lo:hi, :],
                    in_=wk[lo * 128:hi * 128, :].rearrange("(a p) f -> p a f", p=128))
            nc.scalar.dma_start(out=cosf_sb, in_=cosf[:, :])
            nc.scalar.dma_start(out=sins_sb, in_=sins[:, :])
            for qtr in range(4):
                rows = slice(qtr * nq * 128, (qtr + 1) * nq * 128)
                sl = slice(qtr * nq, (qtr + 1) * nq)
                nc.gpsimd.dma_start(
                    out=wv_sb[:, sl, :],
                    in_=wv[rows, :].rearrange("(a p) f -> p a f", p=128))
            nc.gpsimd.dma_start(out=tri_sb, in_=tri[:, :])
            nc.gpsimd.dma_start(
                out=wp_sb, in_=wp[:, :].rearrange("(h p) f -> p h f", p=128))
            nc.vector.memset(ones_bf, 1.0)

            # --- phase 1: qkv projection + RoPE --------------------------
            # one pass over xT per t-chunk: q,k (8 psums), then v (4 psums)
            # from the same SBUF-resident x chunk.
            def rope4(psums, dest, tcn):
                """4 head psums [128,512] f32 -> dest[:, :, tc] rotated bf16."""
                tmp4 = work.tile([128, HPC, 512], BF, tag="rope_tmp", bufs=2,
                                 name="tmp4")
                for h in range(HPC):
                    nc.scalar.copy(out=tmp4[:, h, :], in_=psums[h])
                sw4 = work.tile([128, HPC, 512], BF, tag="rope_sw", bufs=2,
                                name="sw4")
                nc.scalar.dma_start(out=sw4[0:64, :, :], in_=tmp4[64:128, :, :])
                nc.scalar.dma_start(out=sw4[64:128, :, :], in_=tmp4[0:64, :, :])
                cosb = bcast_head(cosf_sb[:, tcn * 512:(tcn + 1) * 512])
                sinb = bcast_head(sins_sb[:, tcn * 512:(tcn + 1) * 512])
                nc.vector.tensor_mul(tmp4, tmp4, cosb)
                nc.vector.tensor_mul(sw4, sw4, sinb)
                nc.vector.tensor_add(
                    dest[:, :, tcn * 512:(tcn + 1) * 512], tmp4, sw4)

            with tc.tile_pool(name="ps1", bufs=8, space="PSUM") as ps1:
                nh = n_ct // 2
                for tcn in range(n_tc):
                    xtc_halves = []
                    nsplit = 2
                    for half in range(2):
                        xh = work.tile([128, nh, 512], BF, tag="xtc", bufs=3,
                                       name=f"xtc{tcn}_{half}")
                        step = nh // nsplit
                        for qtr in range(nsplit):
                            a0 = half * nh + qtr * step
                            rows = slice(a0 * 128, (a0 + step) * 128)
                            sl = slice(qtr * step, (qtr + 1) * step)
                            nc.sync.dma_start(
                                out=xh[:, sl, :],
                                in_=xT[rows, tcn * 512:(tcn + 1) * 512]
                                .rearrange("(a p) t -> p a t", p=128))
                        xtc_halves.append(xh)
                    def xct(ct):
                        return xtc_halves[ct // nh][:, ct % nh, :]
                    pq = [ps1.tile([128, 512], F32, tag="p1", name=f"pq{tcn}_{i}")
                          for i in range(HPC)]
                    pk = [ps1.tile([128, 512], F32, tag="p1", name=f"pk{tcn}_{i}")
                          for i in range(HPC)]
                    for ct in range(n_ct):
                        for h in range(HPC):
                            nc.tensor.matmul(
                                pq[h], wq_sb[:, ct, h * D:(h + 1) * D], xct(ct),
                                start=(ct == 0), stop=(ct == n_ct - 1))
                            nc.tensor.matmul(
                                pk[h], wk_sb[:, ct, h * D:(h + 1) * D], xct(ct),
                                start=(ct == 0), stop=(ct == n_ct - 1))
                    rope4(pq, qrot, tcn)
                    rope4(pk, krot, tcn)

                    # v for this t-chunk, from the same resident x chunk
                    pv = [ps1.tile([128, DV], F32, tag="p1", name=f"pv{tcn}_{i}")
                          for i in range(4)]
                    for ct in range(n_ct):
                        for ts in range(4):
                            nc.tensor.matmul(
                                pv[ts], xct(ct)[:, ts * 128:(ts + 1) * 128],
                                wv_sb[:, ct, :],
                                start=(ct == 0), stop=(ct == n_ct - 1))
                    for ts in range(4):
                        nc.scalar.copy(out=v_sb[:, tcn * 4 + ts, :], in_=pv[ts])

            # --- phase 2+3: attention + projection -----------------------
            import concourse.bass as bass
            with tc.tile_pool(name="dram", bufs=2, space="DRAM") as dram, \
                 tc.tile_pool(name="ps2", bufs=1, space="PSUM") as ps2:
                def proj_block(qc):
                    for ti in range(qc * 4, qc * 4 + 4):
                        y_sb = work.tile([128, C], F32, tag="y_sb", bufs=2,
                                         name="y_sb")
                        for cc in range(C // 512):
                            py = ps2.tile([128, 512], F32, tag="y", bufs=YBUFS,
                                          name=f"py{ti}_{cc}")
                            for h in range(HPC):
                                nc.tensor.matmul(
                                    py, attoT[:, h, ti * 128:(ti + 1) * 128],
                                    wp_sb[:, h, cc * 512:(cc + 1) * 512],
                                    start=(h == 0), stop=(h == HPC - 1))
                            if cc % 2 == 0:
                                nc.scalar.copy(out=y_sb[:, cc * 512:(cc + 1) * 512], in_=py)
                            else:
                                nc.vector.tensor_copy(out=y_sb[:, cc * 512:(cc + 1) * 512], in_=py)
                        nc.scalar.dma_start(
                            out=y[ti * 128:(ti + 1) * 128, :], in_=y_sb)

                for qc in range(n_qc):
                    for h in range(HPC):
                        nkt = 4 * qc + 4
                        po = ps2.tile([128, 512], F32, tag="o", bufs=2, name=f"po{qc}_{h}")
                        pz = ps2.tile([1, 512], F32, tag="z", bufs=ZBUFS, name=f"pz{qc}_{h}")
                        for kt in range(nkt):
                            r = kt - 4 * qc
                            off = 128 * r if r >= 0 else 0
                            N = 512 - off
                            ps_s = ps2.tile([128, 512], F32, tag="s", bufs=SBUFS,
                                            name=f"s{qc}_{h}_{kt}")
                            nc.tensor.matmul(
                                ps_s[:, :N],
                                krot[:, h, kt * 128:(kt + 1) * 128],
                                qrot[:, h, qc * 512 + off:(qc + 1) * 512],
                                start=True, stop=True)
                            expT = work.tile([128, 512], BF, tag="expT", bufs=4,
                                             name="expT")
                            nc.scalar.activation(
                                out=expT[:, :N], in_=ps_s[:, :N],
                                func=mybir.ActivationFunctionType.Exp,
                                scale=SCALE)
                            if r >= 0:
                                nc.vector.tensor_mul(
                                    expT[:, 0:128], expT[:, 0:128], tri_sb)
                            nc.tensor.matmul(
                                po[:, off:], v_sb[:, kt, h * D:(h + 1) * D],
                                expT[:, :N],
                                start=(kt == 0), stop=(kt == nkt - 1))
                            nc.tensor.matmul(
                                pz[:, off:], ones_bf, expT[:, :N],
                                start=(kt == 0), stop=(kt == nkt - 1))
                        recipz = work.tile([1, 512], F32, tag="recipz", bufs=2,
                                           name="recipz")
                        nc.vector.reciprocal_approx_fast(out=recipz, in_=pz)
                        # broadcast 1/Z to all partitions via a DRAM bounce on
                        # the (idle in this phase) sync queue: SBUF->DRAM 2KB,
                        # then a partition-broadcast DRAM->SBUF read.
                        zdram = dram.tile([1, 512], F32, tag="zd", bufs=2,
                                          name=f"zd{qc}_{h}")
                        nc.sync.dma_start(out=zdram, in_=recipz)
                        zb_sb = work.tile([128, 512], F32, tag="zb_sb", bufs=2,
                                          name="zb_sb")
                        zb_bc = bass.AP(tensor=zdram.tensor, offset=zdram.offset,
                                        ap=[[0, 128]] + list(zdram.ap[1:]))
                        nc.sync.dma_start(out=zb_sb, in_=zb_bc)
                        nc.vector.tensor_mul(
                            attoT[:, h, qc * 512:(qc + 1) * 512], po, zb_sb)

                    if qc > 0:
                        proj_block(qc - 1)
                proj_block(n_qc - 1)

    nc.compile()
    return nc


_NC_CACHE = None


def get_nc():
    global _NC_CACHE
    if _NC_CACHE is None:
        _NC_CACHE = build_nc()
    return _NC_CACHE


def make_in_maps(x, w_attn, w_proj, start_index):
    """Full inputs -> per-core in_maps (core c: batch c//4, head group c%4)."""
    x = np.asarray(x, dtype=np.float32)
    w_attn = np.asarray(w_attn, dtype=np.float32)
    w_proj = np.asarray(w_proj, dtype=np.float32)
    si = int(np.asarray(start_index).item()) if np.asarray(start_index).shape == () \
        else int(start_index)

    wq_full = w_attn[:, 0 * C:1 * C]
    wk_full = w_attn[:, 1 * C:2 * C]
    wv_full = w_attn[:, 2 * C:3 * C]

    in_maps = []
    for c in range(N_CORES):
        b, g = divmod(c, GROUPS)
        cols = slice(g * DV, (g + 1) * DV)
        in_maps.append(make_core_inputs(
            x[b], wq_full[:, cols], wk_full[:, cols], wv_full[:, cols],
            w_proj[g * DV:(g + 1) * DV, :], si))
    return in_maps


def kernel(x, w_attn, w_proj, start_index):
    nc = get_nc()
    in_maps = make_in_maps(x, w_attn, w_proj, start_index)
    res = run_bass_kernel_spmd(nc, in_maps, core_ids=list(range(N_CORES)))
    out = np.zeros((B, T, C), dtype=np.float32)
    for c in range(N_CORES):
        b = c // GROUPS
        out[b] += res.results[c]["y"]
    return out
